# revision 10
# baseline (speedup 1.0000x reference)
"""AttentionSubsample kernel for 8 trn2 NeuronCores (v2).

Sharding: head-parallel (8 heads -> 8 cores); final projection sharded by
output channels after an AllGather of per-head attention outputs.

v2 changes vs baseline (cost-model driven):
- Relative-position bias is added INTO the QK PSUM by a cheap fp8e4m3
  DoubleRow matmul (identity stationary, bias moving; 0.5 cyc/row) instead
  of multiplying exp(bias) on DVE. Removes ~50us of DVE work and halves
  the bias HBM traffic. Softmax scale is folded into the q BN gains
  host-side, so ACT exp reads PSUM directly with no extra work.
- attn@V runs "swapped": attention chunk as stationary, V as moving, so
  the matmul's moving size is 33 (DV+ones) instead of 448. The attention
  matrix for one (batch, q-chunk) stays resident in SBUF (bf16) and the
  42-k-tile accumulation chains run region-sequential in one PSUM bank
  (interleaved open accumulation groups in a bank are broken on HW).
- Output of attn@V is token-major [q, dv]; softmax denominators come from
  the ones-column and are applied as per-partition tensor_scalar scalars
  (no PE broadcast matmul). hardswish on DVE, then PE transposes produce
  the channel-major hsT for the AllGather + projection (as baseline).
- Projection / BN psum->sbuf copies moved to the GPSIMD (Pool) engine so
  ACT does exp only.
- x / xs are streamed in chunks (no big front-loaded DMA).
"""

import numpy as np
import ml_dtypes

import concourse.bass as bass
import concourse.mybir as mybir
import concourse.tile as tile
from concourse import bacc
from contextlib import ExitStack
from concourse.bass_utils import run_bass_kernel_spmd

BF16 = mybir.dt.bfloat16
F32 = mybir.dt.float32
F8 = mybir.dt.float8e4
bf16 = ml_dtypes.bfloat16
f8e4 = ml_dtypes.float8_e4m3

B = 2
ROW, COL = 63, 84
ROW_, COL_ = 32, 42
N = ROW * COL            # 5292 kv tokens
NQ = ROW_ * COL_         # 1344 q tokens
NPAD = 5376              # 42*128 padded kv tokens
KTN = NPAD // 128        # 42 k-tiles
QC = 448                 # q chunk
NQC = NQ // QC           # 3
QQ = 112                 # q sub-chunk (attn@V stationary width)
NQQ = QC // QQ           # 4
CIN = 256
H = 8
KD = 16
DV = 32
HKV = KD + DV            # 48 per-head kv channels
KVP = 64                 # padded kv rows: k at 0:16, v at 32:64
OC = 64                  # per-core slice of the 512 output channels
GRP = 3                  # k-tiles per exp group
NGRP = KTN // GRP        # 14
EPS = 1e-5
SCALE = KD ** -0.5
NCORES = 8
TCH = 448                # x stream chunk (tokens)

LAST_EXEC_NS = None
_prog_cache = {}


def _build_program():
    nc = bacc.Bacc(num_devices=NCORES)

    xT = nc.dram_tensor("xT", [B, 2, 128, NPAD], BF16, kind="ExternalInput")
    xsT = nc.dram_tensor("xsT", [B, 2, 128, NQ], BF16, kind="ExternalInput")
    wkvT = nc.dram_tensor("wkvT", [2, 128, KVP], BF16, kind="ExternalInput")
    wqT = nc.dram_tensor("wqT", [2, 128, KD], BF16, kind="ExternalInput")
    wpT = nc.dram_tensor("wpT", [2, 128, OC], BF16, kind="ExternalInput")
    kv_gb = nc.dram_tensor("kv_gb", [KVP, 2], F32, kind="ExternalInput")
    q_gb = nc.dram_tensor("q_gb", [KD, 2], F32, kind="ExternalInput")
    p_gb = nc.dram_tensor("p_gb", [OC, 2], F32, kind="ExternalInput")
    i2T = nc.dram_tensor("i2T", [128, 2, 128], F8, kind="ExternalInput")
    identT = nc.dram_tensor("identT", [128, 128], BF16, kind="ExternalInput")
    bT = nc.dram_tensor("bT", [NQC, NGRP, 128, (GRP + 1) * QC], F8,
                        kind="ExternalInput")
    yT = nc.dram_tensor("yT", [OC, B * NQ], F32, kind="ExternalOutput")

    with ExitStack() as ctx:
        tc = ctx.enter_context(tile.TileContext(nc))
        const = ctx.enter_context(tc.tile_pool(name="const", bufs=1))
        big = ctx.enter_context(tc.tile_pool(name="big", bufs=1))
        xstr = ctx.enter_context(tc.tile_pool(name="xstr", bufs=3))
        vtp = ctx.enter_context(tc.tile_pool(name="vtp", bufs=1))
        bpool = ctx.enter_context(tc.tile_pool(name="bpool", bufs=3))
        small = ctx.enter_context(tc.tile_pool(name="small", bufs=4))
        drain = ctx.enter_context(tc.tile_pool(name="drain", bufs=3))
        dram = ctx.enter_context(tc.tile_pool(name="dram", bufs=4, space="DRAM"))

        mult = mybir.AluOpType.mult
        add = mybir.AluOpType.add
        amin = mybir.AluOpType.min
        Act = mybir.ActivationFunctionType
        DR = mybir.MatmulPerfMode.DoubleRow

        # ------------------------- consts -------------------------
        wkv_sb = const.tile([128, 2, KVP], BF16, tag="wkv")
        wq_sb = const.tile([128, 2, KD], BF16, tag="wq")
        wp_sb = const.tile([128, 2, OC], BF16, tag="wp")
        for c in range(2):
            nc.sync.dma_start(out=wkv_sb[:, c, :], in_=wkvT[c])
            nc.sync.dma_start(out=wq_sb[:, c, :], in_=wqT[c])
            nc.sync.dma_start(out=wp_sb[:, c, :], in_=wpT[c])
        kvgb_sb = const.tile([KVP, 2], F32, tag="kvgb")
        qgb_sb = const.tile([KD, 2], F32, tag="qgb")
        pgb_sb = const.tile([OC, 2], F32, tag="pgb")
        nc.sync.dma_start(out=kvgb_sb, in_=kv_gb[:, :])
        nc.sync.dma_start(out=qgb_sb, in_=q_gb[:, :])
        nc.sync.dma_start(out=pgb_sb, in_=p_gb[:, :])
        i2_sb = const.tile([128, 2, 128], F8, tag="i2")
        nc.sync.dma_start(out=i2_sb, in_=i2T[:, :, :])
        ident_sb = const.tile([128, 128], BF16, tag="ident")
        nc.sync.dma_start(out=ident_sb, in_=identT[:, :])
        eps_t = const.tile([128, 1], F32, tag="eps")
        nc.vector.memset(eps_t, EPS)

        # ------------------- projections (streamed) -------------------
        # q first (small, unblocks q BN early)
        y_q = big.tile([KD, B, NQ], BF16, tag="yq")
        st_q = small.tile([KD, 2 * NQC, 6], F32, tag="st_q")
        NT_KV = NPAD // TCH   # 12
        y_kv = big.tile([KVP, B, NPAD], BF16, tag="ykv")
        st_kv = small.tile([KVP, 2 * NT_KV, 6], F32, tag="st_kv")
        with tc.tile_pool(name="psS", bufs=2, space="PSUM") as psS:
            for b in range(B):
                for t in range(NQC):
                    xs_c = xstr.tile([128, 2, QC], BF16, tag="xs")
                    nc.sync.dma_start(
                        out=xs_c, in_=xsT[b, :, :, bass.ts(t, QC)].rearrange(
                            "c p q -> p c q"))
                    ps = psS.tile([KD, TCH], F32, tag="ps_small")
                    for c in range(2):
                        nc.tensor.matmul(ps, wq_sb[:, c, :], xs_c[:, c, :],
                                         start=(c == 0), stop=(c == 1))
                    nc.vector.tensor_copy(y_q[:, b, bass.ts(t, QC)], ps)
                    nc.vector.bn_stats(out=st_q[:, b * NQC + t, :],
                                       in_=y_q[:, b, bass.ts(t, QC)])

            for b in range(B):
                for t in range(NT_KV):
                    xt_c = xstr.tile([128, 2, TCH], BF16, tag="xt")
                    nc.sync.dma_start(
                        out=xt_c, in_=xT[b, :, :, bass.ts(t, TCH)].rearrange(
                            "c p q -> p c q"))
                    ps = psS.tile([KVP, TCH], F32, tag="ps_kv")
                    for c in range(2):
                        nc.tensor.matmul(ps, wkv_sb[:, c, :], xt_c[:, c, :],
                                         start=(c == 0), stop=(c == 1))
                    nc.vector.tensor_copy(y_kv[:, b, bass.ts(t, TCH)], ps)
                    # stats over the real token range only: 5292 = 12*441
                    nc.vector.bn_stats(out=st_kv[:, b * NT_KV + t, :],
                                       in_=y_kv[:, b, bass.ds(t * 441, 441)])

        # ------------------------- batch norms -------------------------
        def bn_scale_shift(mv, gb, P, name):
            s = small.tile([P, 1], F32, tag=f"s_{name}", name=f"s_{name}")
            t = small.tile([P, 1], F32, tag=f"t_{name}", name=f"t_{name}")
            nc.scalar.activation(out=s, in_=mv[:, 1:2], func=Act.Sqrt,
                                 bias=eps_t[0:P])
            nc.vector.reciprocal(out=s, in_=s)
            nc.vector.tensor_mul(s, s, gb[:, 0:1])
            nc.vector.tensor_mul(t, mv[:, 0:1], s)
            nc.vector.tensor_scalar(out=t, in0=t, scalar1=-1.0, scalar2=None,
                                    op0=mult)
            nc.vector.tensor_add(t, t, gb[:, 1:2])
            return s, t

        mv_q = small.tile([KD, 2], F32, tag="mv_q")
        nc.vector.bn_aggr(out=mv_q, in_=st_q)
        s_q, t_q = bn_scale_shift(mv_q, qgb_sb, KD, "q")

        mv_kv = small.tile([KVP, 2], F32, tag="mv_kv")
        nc.vector.bn_aggr(out=mv_kv, in_=st_kv)
        s_kv, t_kv = bn_scale_shift(mv_kv, kvgb_sb, KVP, "kv")

        # normalized kT, qT (bf16); v token-major with ones column
        kT = big.tile([KD, B, NPAD], BF16, tag="kT")
        qT = big.tile([KD, B, NQ], BF16, tag="qT")
        v_aug = big.tile([128, B, KTN, DV + 1], BF16, tag="vaug")
        for b in range(B):
            nc.vector.tensor_scalar(out=kT[:, b, :], in0=y_kv[0:KD, b, :],
                                    scalar1=s_kv[0:KD], scalar2=t_kv[0:KD],
                                    op0=mult, op1=add)
            nc.vector.tensor_scalar(out=qT[:, b, :], in0=y_q[:, b, :],
                                    scalar1=s_q, scalar2=t_q,
                                    op0=mult, op1=add)
        for b in range(B):
            vTn = vtp.tile([DV, NPAD], BF16, tag="vTn")
            nc.vector.tensor_scalar(out=vTn, in0=y_kv[32:KVP, b, :],
                                    scalar1=s_kv[32:KVP], scalar2=t_kv[32:KVP],
                                    op0=mult, op1=add)
            vtd = vtp.tile([128, KTN, DV], BF16, tag="vtd")
            nc.sync.dma_start_transpose(out=vtd, in_=vTn)
            nc.vector.tensor_copy(v_aug[:, b, :, 0:DV], vtd)
            nc.vector.memset(v_aug[:, b, :, DV:DV + 1], 1.0)

        # ------------------------- attention -------------------------
        # sp buffers: one full attention strip per batch, reused across qc.
        sp0 = big.tile([128, KTN, QC], BF16, tag="sp0")
        sp1 = big.tile([128, KTN, QC], BF16, tag="sp1")
        sps = [sp0, sp1]
        hsT = big.tile([DV, B, NQ], BF16, tag="hsT")
        hs_bounce = dram.tile([NQC, DV, B * QC], BF16, tag="hs_bounce")
        hs_all = dram.tile([NQC, H * DV, B * QC], BF16, tag="hs_all")

        attn_ctx = ExitStack()
        psA = attn_ctx.enter_context(
            tc.tile_pool(name="psA", bufs=2, space="PSUM"))
        psB = attn_ctx.enter_context(
            tc.tile_pool(name="psB", bufs=1, space="PSUM"))

        for qc in range(NQC):
            # QK + bias + exp, streaming bias tiles
            for g in range(NGRP):
                b2 = bpool.tile([128, (GRP + 1) * QC], F8, tag="b2")
                nc.sync.dma_start(out=b2, in_=bT[qc, g])
                for b in range(B):
                    # 512-wide regions: each (i) region exactly owns one
                    # PSUM bank -- a start=True clears has_written for the
                    # WHOLE bank, so open accumulation chains must never
                    # share a bank.
                    qk = psA.tile([128, GRP, 512], F32, tag="qk")
                    for i in range(GRP):
                        j = g * GRP + i
                        nc.tensor.matmul(qk[:, i, 0:QC],
                                         kT[:, b, bass.ts(j, 128)],
                                         qT[:, b, bass.ts(qc, QC)],
                                         start=True, stop=False)
                        nc.tensor.matmul(
                            qk[:, i, 0:QC], i2_sb[:, :, :],
                            b2[:, bass.ds(i * QC, 2 * QC)].rearrange(
                                "p (j q) -> p j q", j=2),
                            start=False, stop=True, perf_mode=DR,
                            skip_group_check=True)
                    nc.scalar.activation(
                        out=sps[b][:, bass.ds(g * GRP, GRP), :],
                        in_=qk[:, :, 0:QC], func=Act.Exp)

            # attn@V swapped: sp chunk stationary, V moving; sequential
            # per-region accumulation chains (one psum bank).
            av = psB.tile([QQ, B, NQQ, DV + 1], F32, tag="av")
            tp = psB.tile([DV, B, QC], BF16, tag="tp")
            for b in range(B):
                for qq in range(NQQ):
                    for j in range(KTN):
                        nc.tensor.matmul(av[:, b, qq, :],
                                         sps[b][:, j, bass.ds(qq * QQ, QQ)],
                                         v_aug[:, b, j, :],
                                         start=(j == 0), stop=(j == KTN - 1),
                                         skip_group_check=True)
                # drain: denominators via per-partition scalars + hardswish
                av_sb = drain.tile([QQ, NQQ, DV + 1], F32, tag="av_sb")
                nc.vector.tensor_copy(av_sb, av[:, b, :, :])
                rec = drain.tile([QQ, NQQ, 1], F32, tag="rec")
                nc.vector.reciprocal(out=rec, in_=av_sb[:, :, DV:DV + 1])
                xo = drain.tile([QQ, NQQ, DV], F32, tag="xo")
                for qq in range(NQQ):
                    nc.vector.tensor_scalar(out=xo[:, qq, :],
                                            in0=av_sb[:, qq, 0:DV],
                                            scalar1=rec[:, qq, :],
                                            scalar2=None, op0=mult)
                r3 = drain.tile([QQ, NQQ, DV], F32, tag="r3")
                nc.vector.tensor_scalar(out=r3, in0=xo, scalar1=3.0,
                                        scalar2=0.0, op0=add,
                                        op1=mybir.AluOpType.max)
                nc.vector.tensor_scalar(out=r3, in0=r3, scalar1=6.0,
                                        scalar2=1.0 / 6.0, op0=amin, op1=mult)
                hs_tok = drain.tile([QQ, NQQ, DV], BF16, tag="hs_tok")
                nc.vector.tensor_mul(hs_tok, xo, r3)
                for qq in range(NQQ):
                    nc.tensor.transpose(tp[:, b, bass.ds(qq * QQ, QQ)],
                                        hs_tok[:, qq, :],
                                        ident_sb[0:QQ, 0:QQ])
                nc.vector.tensor_copy(hsT[:, b, bass.ts(qc, QC)], tp[:, b, :])
            nc.sync.dma_start(
                out=hs_bounce[qc].rearrange("d (b q) -> d b q", b=B),
                in_=hsT[:, :, bass.ts(qc, QC)])
            nc.gpsimd.collective_compute(
                "AllGather", mybir.AluOpType.bypass,
                replica_groups=[list(range(NCORES))],
                ins=[hs_bounce[qc].opt()],
                outs=[hs_all[qc].opt()])

        attn_ctx.close()

        # --------------------- projection (chunked) ---------------------
        y_p = big.tile([OC, B * NQ], F32, tag="yp")
        st_p = small.tile([OC, B * NQ // QC, 6], F32, tag="st_p")
        with tc.tile_pool(name="psE", bufs=2, space="PSUM") as psE:
            for qc in range(NQC):
                hsall_sb = drain.tile([128, 2, B * QC], BF16, tag="hsall")
                for c in range(2):
                    nc.sync.dma_start(out=hsall_sb[:, c, :],
                                      in_=hs_all[qc, bass.ts(c, 128), :])
                for b in range(B):
                    ps = psE.tile([OC, QC], F32, tag="ps_small")
                    for c in range(2):
                        nc.tensor.matmul(ps, wp_sb[:, c, :],
                                         hsall_sb[:, c, bass.ds(b * QC, QC)],
                                         start=(c == 0), stop=(c == 1))
                    i = b * NQC + qc
                    nc.vector.tensor_copy(
                        y_p[:, bass.ds(b * NQ + qc * QC, QC)], ps)
                    nc.vector.bn_stats(out=st_p[:, i, :],
                                       in_=y_p[:, bass.ds(b * NQ + qc * QC, QC)])
        mv_p = small.tile([OC, 2], F32, tag="mv_p")
        nc.vector.bn_aggr(out=mv_p, in_=st_p)
        s_p, t_p = bn_scale_shift(mv_p, pgb_sb, OC, "p")
        nc.vector.tensor_scalar(out=y_p, in0=y_p, scalar1=s_p, scalar2=t_p,
                                op0=mult, op1=add)
        nc.sync.dma_start(out=yT[:, :], in_=y_p)

    nc.finalize()
    return nc


def _prep_inputs(x, kv_w, kv_g, kv_b, q_w, q_g, q_b, proj_w, proj_g, proj_b,
                 bias_table, bias_idxs):
    """Host-side sharding/layout prep. Returns list of 8 per-core input maps."""
    x = np.asarray(x, np.float32)
    xt = np.zeros((B, 2, 128, NPAD), np.float32)
    xTt = x.transpose(0, 2, 1)  # (B, 256, N)
    xt[:, :, :, :N] = xTt.reshape(B, 2, 128, N)
    xt = xt.astype(bf16)
    xs = x.reshape(B, ROW, COL, CIN)[:, ::2, ::2].reshape(B, NQ, CIN)
    xst = xs.transpose(0, 2, 1).reshape(B, 2, 128, NQ).astype(bf16)

    # raw bias (not exp), padded-k rows get -32 (dead after exp)
    rank2 = np.asarray(bias_idxs)[0].reshape(ROW, COL)
    table2 = np.asarray(bias_table, np.float32)[:, rank2]  # (H, 63, 84)
    kk = np.arange(N)
    qq = np.arange(NQ)
    DRm = np.abs(kk[:, None] // COL - 2 * (qq[None, :] // COL_))
    DCm = np.abs(kk[:, None] % COL - 2 * (qq[None, :] % COL_))

    i2 = np.zeros((128, 2, 128), np.float32)
    i2[:, 0, :] = np.eye(128)
    i2 = i2.astype(f8e4)
    ident = np.eye(128, dtype=np.float32).astype(bf16)

    in_maps = []
    for h in range(H):
        bfull = np.full((NPAD, NQ), -32.0, np.float32)
        bfull[:N] = table2[h][DRm, DCm]
        # (NPAD, NQ) -> (NQC, NGRP, 128, (GRP+1)*QC) with next-tile slop
        bk = bfull.reshape(KTN, 128, NQC, QC)       # (ktile, p, qc, q)
        bl = np.zeros((NQC, NGRP, 128, (GRP + 1) * QC), np.float32)
        for g in range(NGRP):
            for i in range(GRP + 1):
                j = g * GRP + i
                if j < KTN:
                    bl[:, g, :, i * QC:(i + 1) * QC] = bk[j].transpose(1, 0, 2)
        blf = bl.astype(f8e4)

        sl = slice(h * HKV, (h + 1) * HKV)
        slq = slice(h * KD, (h + 1) * KD)
        slo = slice(h * OC, (h + 1) * OC)
        wkv_pad = np.zeros((KVP, CIN), np.float32)
        wkv_pad[0:KD] = np.asarray(kv_w, np.float32)[sl][0:KD]
        wkv_pad[32:KVP] = np.asarray(kv_w, np.float32)[sl][KD:HKV]
        kvgb_pad = np.zeros((KVP, 2), np.float32)
        kvgb_pad[:, 0] = 1.0
        kvgb_pad[0:KD, 0] = np.asarray(kv_g, np.float32)[sl][0:KD]
        kvgb_pad[0:KD, 1] = np.asarray(kv_b, np.float32)[sl][0:KD]
        kvgb_pad[32:KVP, 0] = np.asarray(kv_g, np.float32)[sl][KD:HKV]
        kvgb_pad[32:KVP, 1] = np.asarray(kv_b, np.float32)[sl][KD:HKV]
        in_maps.append({
            "xT": xt,
            "xsT": xst,
            "wkvT": np.ascontiguousarray(
                wkv_pad.T.reshape(2, 128, KVP)).astype(bf16),
            "wqT": np.ascontiguousarray(
                np.asarray(q_w, np.float32)[slq].T.reshape(2, 128, KD)
            ).astype(bf16),
            "wpT": np.ascontiguousarray(
                np.asarray(proj_w, np.float32)[slo].T.reshape(2, 128, OC)
            ).astype(bf16),
            "kv_gb": np.ascontiguousarray(kvgb_pad),
            # SCALE folded into q BN affine => qk psum is pre-scaled
            "q_gb": np.ascontiguousarray(np.stack(
                [np.asarray(q_g, np.float32)[slq] * SCALE,
                 np.asarray(q_b, np.float32)[slq] * SCALE], axis=1)),
            "p_gb": np.ascontiguousarray(np.stack(
                [np.asarray(proj_g, np.float32)[slo],
                 np.asarray(proj_b, np.float32)[slo]], axis=1)),
            "i2T": i2,
            "identT": ident,
            "bT": blf,
        })
    return in_maps


def kernel(x, kv_w, kv_g, kv_b, q_w, q_g, q_b, proj_w, proj_g, proj_b,
           bias_table, bias_idxs, _trace=False):
    global LAST_EXEC_NS
    if "nc" not in _prog_cache:
        _prog_cache["nc"] = _build_program()
    nc = _prog_cache["nc"]
    in_maps = _prep_inputs(x, kv_w, kv_g, kv_b, q_w, q_g, q_b,
                           proj_w, proj_g, proj_b, bias_table, bias_idxs)
    res = run_bass_kernel_spmd(nc, in_maps, core_ids=list(range(NCORES)),
                               trace=_trace)
    LAST_EXEC_NS = res.exec_time_ns
    yts = [np.asarray(r["yT"]) for r in res.results]  # each (OC, B*NQ)
    y = np.concatenate(yts, axis=0)                   # (512, B*NQ)
    return np.ascontiguousarray(
        y.T.reshape(B, NQ, H * OC).astype(np.float32))
